# revision 26
# baseline (speedup 1.0000x reference)
"""NanoLLM (Mamba-ish, d_state=1, d_conv=1) Trainium2 kernel, 8 NeuronCores.

Sharding: core c owns batch c//2, L-half c%2 (T=1024 tokens, one chunk).
Weights replicated and SBUF-resident for all 8 (tied) layers; every matmul
is token-parallel. The L-recurrence runs per-core via tensor_tensor_scan;
the cross-core state handoff (even -> odd core of each batch pair) is a
pairwise AllGather of the final scan state plus a cumprod fixup:
    y = yb3 + s * cf2,   yb3 = hs*cm*sz + D*xi*sz,   cf2 = cp*cm*sz
with s = (peer final state) * parity. SPMD-symmetric, no control flow.

Precision: float32r matmuls (tf32-class, ~1.4e-4/mm measured) for
emb/in/x/dt/out projections; bf16 head; fp32 elementwise + scan.
norm_w and conv_w are folded into in_proj_w on the host; out_norm_w into
head_w. Emulated end-to-end max rel err ~0.8% vs fp64 (budget 2e-2).

Layout: feature-major [128 feat, 1024 tok] tiles. PSUM tiles [128,1024]
span 2 banks; matmuls write 512-wide halves. Activations never spill."""

import numpy as np
from ml_dtypes import bfloat16 as np_bf16

import concourse.bass as bass
import concourse.bacc as bacc
import concourse.mybir as mybir
import concourse.tile as tile
from concourse.bass_utils import run_bass_kernel_spmd

NCORES = 8
B, L, V, EMB, D = 4, 2048, 6400, 512, 768
ED, DT_RANK = 768, 48
XCOLS, BCOL, CCOL, DTCOL = 112, 64, 96, 0  # padded x_proj layout
NLAYERS = 8
T = 1024          # tokens per core
P = 128
W = 1024          # token tile width (one chunk)
H = 512           # psum half width
KD = D // P       # 6
KE = ED // P      # 6
KM = EMB // P     # 4
NVCH = 25         # head vocab chunks of 256
VP = NVCH * 256   # == V, no padding
EPS = 1e-6

f32 = mybir.dt.float32
f32r = mybir.dt.float32r
bf16 = mybir.dt.bfloat16
AX = mybir.AluOpType
AF = mybir.ActivationFunctionType

RG_PAIRS = [[0, 1], [2, 3], [4, 5], [6, 7]]


def build_program(n_layers=NLAYERS):
    nc = bacc.Bacc(
        "TRN2",
        target_bir_lowering=False,
        debug=False,
        enable_asserts=False,
        num_devices=NCORES,
    )

    def inp(name, shape, dt):
        return nc.dram_tensor(name, shape, dt, kind="ExternalInput").ap()

    g = dict(
        e=inp("e", [EMB, T], f32r),
        w_emb=inp("w_emb", [KM, P, D], f32r),
        w_in=inp("w_in", [KD, P, 2 * ED], f32r),
        w_out=inp("w_out", [KE, P, D], f32r),
        w_x=inp("w_x", [KE, P, XCOLS], f32r),
        w_dt=inp("w_dt", [DT_RANK, ED], bf16),
        w_head=inp("w_head", [NVCH, KD, P, 256], f32r),
        # packed per-channel params [768, 8]: 0 conv_b, 1 dt_b, 2 A0, 3 D
        pp=inp("pp", [D, 8], f32),
        parity=inp("parity", [P, 1], f32),
        out=nc.dram_tensor("out", [T, V], f32, kind="ExternalOutput").ap(),
        cc_in=[
            nc.dram_tensor(f"cc_in{k}", [P, KE], f32, kind="Internal").ap()
            for k in range(n_layers)
        ],
        cc_out=[
            nc.dram_tensor(f"cc_out{k}", [2, P, KE], f32,
                           kind="Internal").ap()
            for k in range(n_layers)
        ],
    )

    with tile.TileContext(nc) as tc:
        _body(nc, tc, g, n_layers)

    nc.compile()
    return nc


def _body(nc, tc, g, n_layers):
    from contextlib import ExitStack

    with ExitStack() as ctx:
        cpool = ctx.enter_context(tc.tile_pool(name="consts", bufs=1))
        ap_ = ctx.enter_context(tc.tile_pool(name="act", bufs=1))
        psp = ctx.enter_context(
            tc.tile_pool(name="ps", bufs=1, space=bass.MemorySpace.PSUM))

        dma = nc.sync.dma_start
        uid = [0]

        def fresh(tag, shape=(P, W), dt=f32, bufs=1, pool=None):
            uid[0] += 1
            return (pool or ap_).tile(
                list(shape), dt, name=f"t{uid[0]}_{tag}", tag=tag, bufs=bufs)

        # ---------------- constants ----------------
        ones_bf = fresh("ones_bf", (P, 1), bf16, pool=cpool)
        nc.vector.memset(ones_bf, 1.0)
        ones_f = fresh("ones_f", (65, P), f32, pool=cpool)
        nc.vector.memset(ones_f, 1.0)
        nc.vector.memset(ones_f[32:33, :], -1.0)
        ones_c = fresh("ones_c", (65, P), f32r, pool=cpool)
        nc.vector.tensor_copy(ones_c, ones_f)
        ones_r = ones_c[0:1, :]
        par_sb = fresh("par_sb", (P, 1), f32, pool=cpool)
        dma(par_sb, g["parity"])
        epsc = fresh("epsc", (1, 1), f32, pool=cpool)
        nc.vector.memset(epsc, EPS)
        pp_sb = []
        for d in range(KD):
            t = fresh(f"pp_{d}", (P, 8), f32, pool=cpool)
            dma(t, g["pp"][d * P:(d + 1) * P, :])
            pp_sb.append(t)
        CB = [t[:, 0:1] for t in pp_sb]
        NDTB = [t[:, 1:2] for t in pp_sb]
        DPr = [t[:, 3:4] for t in pp_sb]


        # resident weights (f32r)
        w_in_sb = []
        for k in range(KD):
            t = fresh(f"w_in_{k}", (P, 2 * ED), f32r, pool=cpool)
            dma(t, g["w_in"][k])
            w_in_sb.append(t)
        w_out_sb = []
        for k in range(KE):
            t = fresh(f"w_out_{k}", (P, D), f32r, pool=cpool)
            dma(t, g["w_out"][k])
            w_out_sb.append(t)
        w_x_sb = []
        for k in range(KE):
            t = fresh(f"w_x_{k}", (P, XCOLS), f32r, pool=cpool)
            dma(t, g["w_x"][k])
            w_x_sb.append(t)
        w_dt_sb = fresh("w_dt", (DT_RANK, ED), bf16, pool=cpool)
        dma(w_dt_sb, g["w_dt"])
        # rstd/bm/cm rows at base partitions 0/32/64 (matmul rhs rule)
        rows = fresh("rows", (65, W), f32r, pool=cpool)

        snd = fresh("snd", (P, KE), f32, pool=cpool)
        srecv = fresh("srecv", (P, KE), f32, bufs=2)
        smask = fresh("smask", (P, KE), f32, bufs=2)

        # ---------------- ring-tag helpers ----------------
        # xy:   e(4) -> x(6) -> yb3(6) -> x(6) ...        [P,W] f32r bufs=6
        # hc:   w_emb(4) -> h(6) -> cf2(6) -> h(6) ...    [P,W] f32r bufs=6
        # xiy:  xi(6) -> y(6) -> xi(6) ...                [P,W] f32r bufs=6
        # sz:   sz(6) per layer                           [P,W] f32  bufs=6
        # tmp:  delta,at,bt,bt2,hs,cp,t1,v,w per m        [P,W] f32  bufs=5
        # bc:   bm_sb, cm_sb per layer                    [P,W] f32  bufs=2
        # sqon: sq(6) per layer -> on(6) at head          [P,W] bf16 bufs=6
        # wk:   head weight stream                        [P,512] bf16 bufs=6
        def xy(nm):
            return fresh("xy", dt=f32r, bufs=6)

        def hc(nm):
            return fresh("hc", dt=f32r, bufs=6)

        def xiy(nm):
            return fresh("xiy", dt=f32r, bufs=6)

        # psum pools: pa 3x[P,W] (6 banks), pb 1x[P,512] (1 bank),
        # wp 1x[16,16] keep-warm (1 bank)
        def pa():
            return fresh("pa", (P, W), f32, bufs=3, pool=psp)

        def pb():
            return fresh("pb", (P, H), f32, bufs=1, pool=psp)

        warm_ps = fresh("wp", (16, 16), f32, bufs=1, pool=psp)

        def keep_warm(src_tile):
            # tiny fp32 matmul dependent on a scan-phase tile: fires midway
            # through the long no-matmul window so the PE HAM stays warm
            nc.tensor.matmul(warm_ps, src_tile[0:1, 0:16],
                             src_tile[0:1, 16:32], start=True, stop=True)

        halves = [(0, slice(0, H)), (1, slice(H, W))]

        def norm_and_h(x_tiles, dst_h):
            """rmsnorm: sq -> stat matmul -> rstd row -> broadcast -> h."""
            sqs = []
            for d in range(KD):
                t = fresh("sq", dt=bf16, bufs=2)
                nc.scalar.square(t, x_tiles[d])
                keep_warm(t)
                sqs.append(t)
            nps = pa()
            for hi, hsl in halves:
                for d in range(KD):
                    nc.tensor.matmul(nps[0:1, hsl], ones_bf, sqs[d][:, hsl],
                                     start=(d == 0), stop=(d == KD - 1))
            # rstd = exp(-0.5*ln(S/D + eps))  (ln/exp table set)
            lr = fresh("tmp", bufs=5)
            for hi, hsl in halves:
                nc.scalar.activation(lr[0:1, hsl], nps[0:1, hsl], AF.Ln,
                                     bias=epsc, scale=1.0 / D)
            nc.scalar.activation(rows[0:1, :], lr[0:1, :], AF.Exp, scale=-0.5)
            rep = pa()
            for hi, hsl in halves:
                nc.tensor.matmul(rep[:, hsl], ones_r, rows[0:1, hsl],
                                 start=True, stop=True)
            rstd_sb = fresh("tmp", bufs=5)
            for hi, hsl in halves:
                nc.scalar.copy(rstd_sb[:, hsl], rep[:, hsl])
            keep_warm(rstd_sb)
            hh = []
            for d in range(KD):
                t = dst_h(d)
                nc.vector.tensor_mul(t, x_tiles[d], rstd_sb)
                hh.append(t)
            return hh

        def proj_out_x(y_or_e, w_sb, nk):
            """contract y tiles (rhs) with w (lhsT cols) -> x tiles."""
            xs = []
            for d in range(KD):
                ps = pa()
                for hi, hsl in halves:
                    for k in range(nk):
                        nc.tensor.matmul(
                            ps[:, hsl],
                            w_sb[k][:, d * P:(d + 1) * P],
                            y_or_e[k][:, hsl],
                            start=(k == 0), stop=(k == nk - 1))
                xt = xy(f"x{d}")
                for hi, hsl in halves:
                    nc.scalar.copy(xt[:, hsl], ps[:, hsl])
                keep_warm(xt)
                xs.append(xt)
            return xs

        # ---------------- embedding projection ----------------
        e_sb = []
        for k in range(KM):
            t = xiy(f"e{k}")
            dma(t, g["e"][k * P:(k + 1) * P, :])
            e_sb.append(t)
        w_emb_sb = []
        for k in range(KM):
            t = hc(f"we{k}")
            dma(t[:, 0:D], g["w_emb"][k])
            w_emb_sb.append(t)
        xs = proj_out_x(e_sb, w_emb_sb, KM)

        # ---------------- layers ----------------
        for ly in range(n_layers):
            hh = norm_and_h(xs, lambda d: hc(f"h{d}"))

            # in_proj xi half (m 0..5)
            xi = []
            for m in range(KE):
                ps = pa()
                for hi, hsl in halves:
                    for k in range(KD):
                        nc.tensor.matmul(
                            ps[:, hsl],
                            w_in_sb[k][:, m * P:(m + 1) * P],
                            hh[k][:, hsl],
                            start=(k == 0), stop=(k == KD - 1))
                t = xiy(f"xi{m}")
                for hi, hsl in halves:
                    nc.scalar.activation(t[:, hsl], ps[:, hsl], AF.Silu,
                                         bias=CB[m])
                keep_warm(t)
                xi.append(t)

            # x_proj -> dbc psum (per half); dtr/bm/cm rows
            dtr = fresh("dtr", (DT_RANK, W), bf16, bufs=1)
            for hi, hsl in halves:
                dps = pb()
                for k in range(KE):
                    nc.tensor.matmul(dps[0:XCOLS, :], w_x_sb[k],
                                     xi[k][:, hsl],
                                     start=(k == 0), stop=(k == KE - 1))
                nc.vector.tensor_copy(dtr[:, hsl],
                                      dps[DTCOL:DTCOL + DT_RANK, :])
                nc.vector.tensor_copy(rows[32:33, hsl],
                                      dps[BCOL:BCOL + 1, :])
                nc.vector.tensor_copy(rows[64:65, hsl],
                                      dps[CCOL:CCOL + 1, :])
            # broadcast bm/cm rows to [P, W] in sbuf
            bm_sb = fresh("bc", dt=bf16, bufs=2)
            cm_sb = fresh("bc", dt=bf16, bufs=2)
            for row, dst in ((32, bm_sb), (64, cm_sb)):
                for hi, hsl in halves:
                    rep = pb()
                    nc.tensor.matmul(rep, ones_c[row:row + 1, :],
                                     rows[row:row + 1, hsl],
                                     start=True, stop=True)
                    nc.scalar.copy(dst[:, hsl], rep)

            # in_proj z half (m 6..11) -> sz
            sz = []
            for m in range(KE):
                ps = pa()
                for hi, hsl in halves:
                    for k in range(KD):
                        nc.tensor.matmul(
                            ps[:, hsl],
                            w_in_sb[k][:, (KE + m) * P:(KE + m + 1) * P],
                            hh[k][:, hsl],
                            start=(k == 0), stop=(k == KD - 1))
                t = fresh("sz", bufs=6)
                for hi, hsl in halves:
                    nc.scalar.activation(t[:, hsl], ps[:, hsl], AF.Silu)
                keep_warm(t)
                sz.append(t)

            # dt -> at=sigmoid(-u-dtb) -> delta_neg=ln(at) -> scan -> y-pre
            # (A0 == -1 structurally: at = exp(-softplus(u+dtb)))
            # groups of 2 m: batches sigmoid/ln acts to limit table loads
            yb3 = [None] * KE
            cf2 = [None] * KE
            for gb in range(0, KE, 2):
                ms = (gb, gb + 1)
                ats = []
                for m in ms:
                    ups = pa()
                    for hi, hsl in halves:
                        nc.tensor.matmul(ups[:, hsl],
                                         w_dt_sb[:, m * P:(m + 1) * P],
                                         dtr[:, hsl], start=True, stop=True)
                    at = fresh("at", bufs=2)
                    for hi, hsl in halves:
                        nc.scalar.activation(at[:, hsl], ups[:, hsl],
                                             AF.Sigmoid, bias=NDTB[m],
                                             scale=-1.0)
                    ats.append(at)
                for at, m in zip(ats, ms):
                    keep_warm(at)
                    dn = fresh("tmp", bufs=5)
                    nc.scalar.activation(dn, at, AF.Ln)
                    bt = fresh("tmp", bufs=5)
                    nc.vector.tensor_mul(bt, dn, xi[m])
                    bt2 = fresh("tmp", bufs=5)
                    nc.vector.tensor_mul(bt2, bt, bm_sb)  # bm_sb = -bm
                    hs = fresh("tmp", bufs=5)
                    nc.vector.tensor_tensor_scan(
                        hs[:, 0:H], at[:, 0:H], bt2[:, 0:H], initial=0.0,
                        op0=AX.mult, op1=AX.add)
                    nc.vector.tensor_tensor_scan(
                        hs[:, H:W], at[:, H:W], bt2[:, H:W],
                        initial=hs[:, H - 1:H], op0=AX.mult, op1=AX.add)
                    nc.vector.tensor_copy(snd[:, m:m + 1], hs[:, W - 1:W])
                    keep_warm(hs)
                    # at in (0,1): min(at*state, at) == at*state (cumprod)
                    cp = fresh("tmp", bufs=5)
                    nc.vector.tensor_tensor_scan(
                        cp[:, 0:H], at[:, 0:H], at[:, 0:H], initial=1.0,
                        op0=AX.mult, op1=AX.min)
                    nc.vector.tensor_tensor_scan(
                        cp[:, H:W], at[:, H:W], at[:, H:W],
                        initial=cp[:, H - 1:H], op0=AX.mult, op1=AX.min)
                    if m == KE - 1:
                        dma(g["cc_in"][ly], snd)
                        nc.gpsimd.collective_compute(
                            "AllGather", AX.bypass, replica_groups=RG_PAIRS,
                            ins=[g["cc_in"][ly]], outs=[g["cc_out"][ly]])
                        dma(srecv, g["cc_out"][ly][0])
                    t1 = fresh("tmp", bufs=5)
                    nc.gpsimd.tensor_mul(t1, cm_sb, sz[m])
                    v = fresh("tmp", bufs=5)
                    nc.gpsimd.tensor_mul(v, hs, t1)
                    dxi = fresh("tmp", bufs=5)
                    nc.scalar.mul(dxi, xi[m], DPr[m])
                    w = fresh("tmp", bufs=5)
                    nc.gpsimd.tensor_mul(w, dxi, sz[m])
                    keep_warm(cp)
                    t = xy(f"yb3{m}")
                    nc.vector.tensor_add(t, v, w)
                    yb3[m] = t
                    t = hc(f"cf2{m}")
                    nc.vector.tensor_mul(t, cp, t1)
                    keep_warm(t)
                    cf2[m] = t

            # y = cf2*smask + yb3  (after collective)
            nc.vector.tensor_scalar_mul(smask, srecv, par_sb)
            ys = []
            for m in range(KE):
                t = xiy(f"y{m}")
                nc.vector.scalar_tensor_tensor(t, cf2[m], smask[:, m:m + 1],
                                               yb3[m], AX.mult, AX.add)
                keep_warm(t)
                ys.append(t)

            # out_proj -> next x (+ sq for next norm)
            xs = proj_out_x(ys, w_out_sb, KE)

        # ---------------- output head ----------------
        on = norm_and_h(xs, lambda d: xiy(f"on{d}"))

        for vch in range(NVCH):
            wk = []
            for k in range(KD):
                t = fresh("wk", (P, 256), f32r, bufs=6)
                dma(t, g["w_head"][vch, k])
                wk.append(t)
            for cb in range(T // P):
                ps = pa()
                for k in range(KD):
                    nc.tensor.matmul(
                        ps[:, 0:256],
                        on[k][:, cb * P:(cb + 1) * P],
                        wk[k],
                        start=(k == 0), stop=(k == KD - 1))
                ot = fresh("ot", (P, 256), f32, bufs=3)
                if cb % 2 == 0:
                    nc.scalar.copy(ot, ps[:, 0:256])
                else:
                    nc.vector.tensor_copy(ot, ps[:, 0:256])
                tok0 = cb * P
                dma(g["out"][tok0:tok0 + P, vch * 256:(vch + 1) * 256], ot)


_CACHE = {}


def _get_program(n_layers=NLAYERS):
    if n_layers not in _CACHE:
        _CACHE[n_layers] = build_program(n_layers)
    return _CACHE[n_layers]


def _pad_wx(wx):
    out = np.zeros((ED, XCOLS), np.float32)
    out[:, BCOL] = wx[:, DT_RANK]
    out[:, CCOL] = wx[:, DT_RANK + 1]
    out[:, DTCOL:DTCOL + DT_RANK] = wx[:, :DT_RANK]
    return np.ascontiguousarray(out.reshape(KE, P, XCOLS))


def prep_inputs(tokens, n_layers, emb_table, emb_proj_w, norm_w, in_proj_w,
                conv_w, conv_b, x_proj_w, dt_proj_w, dt_proj_b, A_log,
                D_param, out_proj_w, out_norm_w, head_w, head_b):
    tokens = np.asarray(tokens)
    emb_table = np.asarray(emb_table, np.float32)
    norm_w = np.asarray(norm_w, np.float32)
    conv_w = np.asarray(conv_w, np.float32)
    A0 = -np.exp(np.asarray(A_log, np.float32)[:, 0])
    assert np.allclose(A0, -1.0), "sigmoid-softplus path requires A0 == -1"
    pp = np.stack([
        np.asarray(conv_b, np.float32),
        -np.asarray(dt_proj_b, np.float32),
        A0,
        np.asarray(D_param, np.float32),
        np.zeros(D, np.float32),
        np.zeros(D, np.float32),
        np.zeros(D, np.float32),
        np.zeros(D, np.float32),
    ], axis=1)

    w_in = norm_w[:, None] * np.asarray(in_proj_w, np.float32)
    w_in[:, :ED] = w_in[:, :ED] * conv_w[:, 0][None, :]
    w_head = (np.asarray(out_norm_w, np.float32)[:, None]
              * np.asarray(head_w, np.float32))
    # [D, V] -> [NVCH, KD, P, 256]
    w_head_t = np.ascontiguousarray(
        w_head.reshape(KD, P, NVCH, 256).transpose(2, 0, 1, 3))

    shared = dict(
        w_emb=np.ascontiguousarray(
            np.asarray(emb_proj_w, np.float32).reshape(KM, P, D)),
        w_in=np.ascontiguousarray(w_in.reshape(KD, P, 2 * ED)),
        w_out=np.ascontiguousarray(
            np.asarray(out_proj_w, np.float32).reshape(KE, P, D)),
        w_x=_pad_wx(np.asarray(x_proj_w, np.float32)),
        w_dt=np.ascontiguousarray(
            np.asarray(dt_proj_w, np.float32).astype(np_bf16)),
        w_head=w_head_t,
        pp=np.ascontiguousarray(pp),
    )
    in_maps = []
    for c in range(NCORES):
        b, half = c // 2, c % 2
        tok = tokens[b, half * T:(half + 1) * T]
        m = dict(shared)
        m["e"] = np.ascontiguousarray(emb_table[tok].T)  # [512, 1024]
        m["parity"] = np.full((P, 1), float(half), np.float32)
        in_maps.append(m)
    return in_maps


LAST_RESULT = None


def kernel(**inputs):
    global LAST_RESULT
    n_layers = int(np.asarray(inputs["n_layers"]))
    assert n_layers == NLAYERS, f"hardcoded for {NLAYERS} layers, got {n_layers}"
    nc = _get_program(NLAYERS)
    in_maps = prep_inputs(**inputs)
    res = run_bass_kernel_spmd(nc, in_maps, core_ids=list(range(NCORES)))
    LAST_RESULT = res
    out = np.empty((B, L, V), np.float32)
    for c in range(NCORES):
        b, half = c // 2, c % 2
        out[b, half * T:(half + 1) * T, :] = res.results[c]["out"]
    out += np.asarray(inputs["head_b"], np.float32)[None, None, :]
    return out
